# revision 1
# baseline (speedup 1.0000x reference)
"""ArteryMixer Trainium2 kernel: 8-core data-parallel over tokens.

Per-token math (B=2,S=2048,A=8,R=8,DIM=1024,H=8,HD=128,SC=16):
  qkv = concat(x+emb, res) @ Wqkv.T ; q,k rmsnorm ; k_res roped (folded into W);
  scores=elu(q@k.T/sqrt(HD)) ; mixed = scores@v/16 ; out = x + scale*(mixed@Wproj.T)

Device layout strategy (per core, 512 tokens):
  - All activations kept FEATURE-on-partitions (x.T etc., host pre-transposed).
  - QKV (q,k_art,k_res·Rope) via W-stationary GEMM -> qT/kT slabs (d-part, col=(t,slot)).
  - V via X-stationary GEMM -> v slabs in row layout (rows=(t,slot) on partitions).
  - artery-embed bias folded in as rank-8 extra matmul (one-hot trick).
  - rmsnorm: square (DVE) + gpsimd partition_all_reduce (f32 internal) + ACT ln/exp rsqrt,
    scale folded: rs_q = rsqrt(ssq/128+eps), rs_k = rsqrt(ssq+128*eps) (= rs*HD^-0.5).
  - attention per 16-token group: scoresT = kT_slice.T @ qT_slice (cross products),
    elu via Relu(ACT)+min(exp-1,0), block-diag mask*(1/16) kills cross-token terms.
  - mixedT = v.T @ routeT accumulated art+res -> feature-part layout feeds proj GEMM.
  - proj W-stationary -> projT ; y.T = projT*mixer_scale + x.T ; output stays transposed,
    host un-transposes.
"""

import numpy as np
import ml_dtypes

bf16 = ml_dtypes.bfloat16

HEADS = 8
HD = 128
DIM = 1024
MD = 1024
A = 8
RKV = 8
SC = 16
EPS = 1.1920929e-07
ROPE_BASE = 10000.0
N_CORES = 8
B, S = 2, 2048
TOK_PER_CORE = (B * S) // N_CORES  # 512
BLK_TOK = 64                        # tokens per pipeline block
NB = TOK_PER_CORE // BLK_TOK        # 8 blocks
CPB = BLK_TOK * 8                   # 512 cols per block (token-major, slot-minor)


def _rope_matrix():
    inv_freq = 1.0 / (ROPE_BASE ** (np.arange(0, HD, 2, dtype=np.float64) / HD))
    c, s = np.cos(inv_freq), np.sin(inv_freq)
    Rm = np.zeros((HD, HD), dtype=np.float64)
    i = np.arange(HD // 2)
    # reference _rope: out1 = x1*c + x2*s ; out2 = -x1*s + x2*c
    Rm[i, i] = c
    Rm[i, i + 64] = s
    Rm[i + 64, i] = -s
    Rm[i + 64, i + 64] = c
    return Rm


def build_program(tok_per_core=TOK_PER_CORE, repeat=1):
    import concourse.bass as bass  # noqa
    import concourse.mybir as mybir
    import concourse.tile as tile
    from concourse import bacc
    from concourse import bass_isa

    dt = mybir.dt
    Alu = mybir.AluOpType
    Act = mybir.ActivationFunctionType

    nb = tok_per_core // BLK_TOK
    COLS = tok_per_core * 8

    nc = bacc.Bacc(None, target_bir_lowering=False)

    xt_art = nc.dram_tensor("xt_art", [DIM, COLS], dt.bfloat16, kind="ExternalInput")
    xt_res = nc.dram_tensor("xt_res", [DIM, COLS], dt.bfloat16, kind="ExternalInput")
    wqkv_t = nc.dram_tensor("wqkv_t", [DIM, 3 * MD], dt.bfloat16, kind="ExternalInput")
    wv_t = nc.dram_tensor("wv_t", [DIM, MD], dt.bfloat16, kind="ExternalInput")
    wproj_t = nc.dram_tensor("wproj_t", [MD, DIM], dt.bfloat16, kind="ExternalInput")
    biasqk_d = nc.dram_tensor("biasqk", [128, 128], dt.bfloat16, kind="ExternalInput")
    biasv_d = nc.dram_tensor("biasv", [128, MD], dt.bfloat16, kind="ExternalInput")
    mask_d = nc.dram_tensor("mask", [128, 128], dt.bfloat16, kind="ExternalInput")
    mscale_d = nc.dram_tensor("mscale", [128, 8], dt.float32, kind="ExternalInput")
    out_t = nc.dram_tensor("out_t", [DIM, COLS], dt.bfloat16, kind="ExternalOutput")

    with tile.TileContext(nc) as tc:
        with (
            tc.tile_pool(name="w", bufs=1) as wpool,
            tc.tile_pool(name="x", bufs=2) as xpool,
            tc.tile_pool(name="slab", bufs=2) as spool,
            tc.tile_pool(name="vslab", bufs=1) as vpool,
            tc.tile_pool(name="nrm", bufs=2) as npool,
            tc.tile_pool(name="att", bufs=2) as fpool,
            tc.tile_pool(name="rtp", bufs=3) as rtpool,
            tc.tile_pool(name="y", bufs=2) as ypool,
            tc.tile_pool(name="mm", bufs=2, space="PSUM") as mmpool,
            tc.tile_pool(name="sc", bufs=2, space="PSUM") as scpool,
            tc.tile_pool(name="mx", bufs=1, space="PSUM") as mxpool,
        ):
            # ---- resident weights/constants ----
            wqkv_sb = wpool.tile([128, 8, 3 * MD], dt.bfloat16)
            nc.sync.dma_start(
                wqkv_sb, wqkv_t[:].rearrange("(dc p) f -> p dc f", p=128)
            )
            wv_sb = wpool.tile([128, 8, MD], dt.bfloat16)
            nc.sync.dma_start(wv_sb, wv_t[:].rearrange("(dc p) f -> p dc f", p=128))
            wproj_sb = wpool.tile([128, 8, DIM], dt.bfloat16)
            nc.sync.dma_start(
                wproj_sb, wproj_t[:].rearrange("(mc p) f -> p mc f", p=128)
            )
            biasqk_sb = wpool.tile([128, 16, 8], dt.bfloat16)
            nc.sync.dma_start(biasqk_sb, biasqk_d[:].rearrange("p (fc a) -> p fc a", a=8))
            biasv_sb = wpool.tile([128, MD], dt.bfloat16)
            nc.sync.dma_start(biasv_sb, biasv_d[:])
            mask_sb = wpool.tile([128, 128], dt.bfloat16)
            nc.sync.dma_start(mask_sb, mask_d[:])
            mscale_sb = wpool.tile([128, 8], dt.float32)
            nc.sync.dma_start(mscale_sb, mscale_d[:])
            eps_q = wpool.tile([128, 1], dt.float32)
            nc.vector.memset(eps_q, EPS)
            eps_k = wpool.tile([128, 1], dt.float32)
            nc.vector.memset(eps_k, HD * EPS)

            xa_dram = xt_art[:].rearrange("(dc p) c -> p dc c", p=128)
            xr_dram = xt_res[:].rearrange("(dc p) c -> p dc c", p=128)
            yo_dram = out_t[:].rearrange("(dc p) c -> p dc c", p=128)

            def build_gemm_items(blk):
                """Allocate block tiles + return GEMM work-item closures."""
                c0 = blk * CPB
                xa_h = [xpool.tile([128, 4, CPB], dt.bfloat16, tag=f"xa{i}", name=f"xa{i}")
                        for i in range(2)]
                xr_h = [xpool.tile([128, 4, CPB], dt.bfloat16, tag=f"xr{i}", name=f"xr{i}")
                        for i in range(2)]
                for i in range(2):
                    nc.sync.dma_start(
                        xa_h[i], xa_dram[:, i * 4 : i * 4 + 4, c0 : c0 + CPB]
                    )
                    nc.sync.dma_start(
                        xr_h[i], xr_dram[:, i * 4 : i * 4 + 4, c0 : c0 + CPB]
                    )
                qT = spool.tile([128, 8, CPB], dt.bfloat16, tag="qT")
                kTa = spool.tile([128, 8, CPB], dt.bfloat16, tag="kTa")
                kTr = spool.tile([128, 8, CPB], dt.bfloat16, tag="kTr")
                va = vpool.tile([128, 4, 8, HD], dt.bfloat16, tag="va")
                vr = vpool.tile([128, 4, 8, HD], dt.bfloat16, tag="vr")
                st = dict(xa_h=xa_h, xr_h=xr_h, qT=qT, kTa=kTa, kTr=kTr,
                          va=va, vr=vr, c0=c0)
                slabs = [qT, kTa, kTr]
                items = []

                def qkv_item(fc):
                    def go():
                        ps = mmpool.tile([128, CPB], dt.float32, tag="mmps")
                        halves = xr_h if fc >= 16 else xa_h
                        for dc in range(8):
                            nc.tensor.matmul(
                                ps,
                                wqkv_sb[:, dc, fc * 128 : (fc + 1) * 128],
                                halves[dc // 4][:, dc % 4, :],
                                start=(dc == 0),
                                stop=(dc == 7),
                            )
                        dst = slabs[fc // 8][:, fc % 8, :]
                        nc.scalar.copy(out=dst, in_=ps)
                        if fc < 16:
                            d3 = dst.rearrange("p (t a) -> p t a", a=8)
                            nc.vector.tensor_add(
                                d3,
                                d3,
                                biasqk_sb[:, fc, None, :].to_broadcast(
                                    (128, BLK_TOK, 8)
                                ),
                            )
                    return go

                def v_item(isart, rc, vh):
                    def go():
                        halves, dstv = (xa_h, va) if isart else (xr_h, vr)
                        ps = mmpool.tile([128, 512], dt.float32, tag="mmps")
                        for dc in range(8):
                            nc.tensor.matmul(
                                ps,
                                halves[dc // 4][:, dc % 4, rc * 128 : (rc + 1) * 128],
                                wv_sb[:, dc, vh * 512 : (vh + 1) * 512],
                                start=(dc == 0),
                                stop=(dc == 7),
                            )
                        dv = dstv[:, rc, vh * 4 : (vh + 1) * 4, :]
                        nc.scalar.copy(out=dv, in_=ps)
                        if isart:
                            nc.vector.tensor_add(
                                dv, dv, biasv_sb[:, vh * 512 : (vh + 1) * 512]
                            )
                    return go

                for fc in range(24):
                    items.append(qkv_item(fc))
                for isart in (True, False):
                    for rc in range(4):
                        for vh in range(2):
                            items.append(v_item(isart, rc, vh))
                return st, items

            def build_attn_items(st):
                """Work items for norm + attention + proj of a block."""
                qT, kTa, kTr = st["qT"], st["kTa"], st["kTr"]
                va, vr, xa_h, c0 = st["va"], st["vr"], st["xa_h"], st["c0"]
                items = []

                def norm_item(slab, epsv, scv, hh):
                    def go():
                        with nc.allow_low_precision(
                            reason="all-reduce upcasts internally; bf16 ~0.4%"
                        ):
                            sl = slab[:, hh, :]
                            sq = npool.tile([128, CPB], dt.bfloat16, tag="nsq")
                            nc.vector.tensor_mul(sq, sl, sl)
                            ssq = npool.tile([128, CPB], dt.bfloat16, tag="nssq")
                            nc.gpsimd.partition_all_reduce(
                                ssq, sq, channels=128,
                                reduce_op=bass_isa.ReduceOp.add,
                            )
                            rt = npool.tile([128, CPB], dt.bfloat16, tag="nsq")
                            nc.scalar.activation(
                                rt, ssq, Act.Sqrt, bias=epsv, scale=scv
                            )
                            rs = npool.tile([128, CPB], dt.bfloat16, tag="nssq")
                            nc.vector.reciprocal(rs, rt)
                            nc.vector.tensor_mul(sl, sl, rs)
                    return go

                mixedT = vpool.tile([128, 8, CPB], dt.bfloat16, tag="mixedT")
                st["mixedT"] = mixedT
                routes = {}

                def scores_item(g, half):
                    def go():
                        gsl = slice(g * 128, (g + 1) * 128)
                        kT = kTa if half == 0 else kTr
                        ps = scpool.tile([128, 8, 128], dt.float32, tag="scps")
                        for h in range(8):
                            nc.tensor.matmul(
                                ps[:, h, :], kT[:, h, gsl], qT[:, h, gsl],
                                start=True, stop=True,
                            )
                        esc = fpool.tile([128, 8, 128], dt.bfloat16, tag="ers")
                        rsc = fpool.tile([128, 8, 128], dt.bfloat16, tag="ers")
                        nc.scalar.activation(esc, ps, Act.Exp)
                        nc.scalar.activation(rsc, ps, Act.Relu)
                        # elu = relu(s) + (min(exp(s),1) - 1)
                        nc.vector.tensor_scalar(esc, esc, 1.0, -1.0, Alu.min, Alu.add)
                        nc.vector.tensor_add(esc, rsc, esc)
                        route = rtpool.tile([128, 8, 128], dt.bfloat16, tag="rt")
                        nc.vector.tensor_mul(
                            route, esc,
                            mask_sb[:, None, :].to_broadcast((128, 8, 128)),
                        )
                        routes[(g, half)] = route
                    return go

                def mixed_item(g):
                    def go():
                        gsl = slice(g * 128, (g + 1) * 128)
                        mx = mxpool.tile([128, 8, 128], dt.float32, tag="mxps")
                        for h in range(8):
                            nc.tensor.matmul(
                                mx[:, h, :], va[:, g, h, :],
                                routes[(g, 0)][:, h, :], start=True, stop=False,
                            )
                            nc.tensor.matmul(
                                mx[:, h, :], vr[:, g, h, :],
                                routes[(g, 1)][:, h, :], start=False, stop=True,
                            )
                        nc.scalar.copy(out=mixedT[:, :, gsl], in_=mx)
                    return go

                def proj_item(dc):
                    def go():
                        ps = mmpool.tile([128, CPB], dt.float32, tag="mmps")
                        for h in range(8):
                            nc.tensor.matmul(
                                ps,
                                wproj_sb[:, h, dc * 128 : (dc + 1) * 128],
                                mixedT[:, h, :],
                                start=(h == 0), stop=(h == 7),
                            )
                        yb = ypool.tile([128, CPB], dt.bfloat16, tag="yb")
                        nc.vector.scalar_tensor_tensor(
                            out=yb, in0=ps, scalar=mscale_sb[:, dc : dc + 1],
                            in1=xa_h[dc // 4][:, dc % 4, :],
                            op0=Alu.mult, op1=Alu.add,
                        )
                        nc.sync.dma_start(yo_dram[:, dc, c0 : c0 + CPB], yb)
                    return go

                norm_items = []
                for slab, epsv, scv in (
                    (qT, eps_q, 1.0 / HD),
                    (kTa, eps_k, 1.0),
                    (kTr, eps_k, 1.0),
                ):
                    for hh in range(8):
                        norm_items.append(norm_item(slab, epsv, scv, hh))
                return dict(
                    norm=norm_items,
                    groups=[(scores_item(g, 0), scores_item(g, 1), mixed_item(g))
                            for g in range(4)],
                    proj=[proj_item(dc) for dc in range(8)],
                )

            def merge(attn, gemm):
                """Structured interleave: norm 3:1 with gemms, then per group
                sc,G,sc,G,G,mx,G, then proj 1:2 with gemms; leftovers last."""
                out = []
                gq = list(gemm)

                def g(n):
                    for _ in range(n):
                        if gq:
                            out.append(gq.pop(0))

                if attn is None:
                    return list(gemm)
                for i, it in enumerate(attn["norm"]):
                    out.append(it)
                    if i % 3 == 2:
                        g(1)
                for sc0, sc1, mx in attn["groups"]:
                    out.append(sc0); g(1)
                    out.append(sc1); g(2)
                    out.append(mx); g(1)
                for p in attn["proj"]:
                    out.append(p); g(2)
                out.extend(gq)
                return out

            blklist = [b for _ in range(repeat) for b in range(nb)]
            prev_st = None
            for i in range(len(blklist) + 1):
                gemm_items = []
                if i < len(blklist):
                    st, gemm_items = build_gemm_items(blklist[i])
                attn = build_attn_items(prev_st) if prev_st is not None else None
                for item in merge(attn, gemm_items):
                    item()
                if i < len(blklist):
                    prev_st = st

    nc.compile()
    return nc


def host_prep(x, artery_embed, residual_kv, Wqkv, Wproj, mixer_scale,
              tok_per_core=TOK_PER_CORE, n_cores=N_CORES):
    T = x.shape[0] * x.shape[1]
    x_flat = np.asarray(x, dtype=np.float32).reshape(T, A, DIM)
    res_flat = np.asarray(residual_kv, dtype=np.float32).reshape(T, RKV, DIM)

    Rm = _rope_matrix()
    Wq = np.asarray(Wqkv[0:MD], dtype=np.float64)
    Wk = np.asarray(Wqkv[MD : 2 * MD], dtype=np.float64)
    Wv = np.asarray(Wqkv[2 * MD : 3 * MD], dtype=np.float64)
    Wk_res = np.einsum("de,hec->hdc", Rm, Wk.reshape(HEADS, HD, DIM)).reshape(MD, DIM)

    wqkv_t = np.ascontiguousarray(
        np.concatenate([Wq, Wk, Wk_res], axis=0).T
    ).astype(bf16)
    wv_t = np.ascontiguousarray(Wv.T).astype(bf16)
    wproj_t = np.ascontiguousarray(np.asarray(Wproj, dtype=np.float64).T).astype(bf16)

    emb = np.asarray(artery_embed, dtype=np.float64)
    bias_q = emb @ Wq.T
    bias_k = emb @ Wk.T
    bias_v = emb @ Wv.T
    # biasqk[p, fc*8+a] = bias_cat[a, fc*128+p]
    bias_cat = np.concatenate([bias_q, bias_k], axis=1)  # (8, 2048)
    biasqk = np.ascontiguousarray(
        bias_cat.reshape(8, 16, 128).transpose(2, 1, 0).reshape(128, 128)
    ).astype(bf16)
    biasv = np.ascontiguousarray(np.tile(bias_v, (16, 1))).astype(bf16)

    mask = np.zeros((128, 128), dtype=np.float32)
    for t in range(16):
        mask[t * 8 : (t + 1) * 8, t * 8 : (t + 1) * 8] = 1.0 / SC
    mask = mask.astype(bf16)

    mscale = np.ascontiguousarray(
        np.asarray(mixer_scale, dtype=np.float32).reshape(8, 128).T
    )

    shared = dict(
        wqkv_t=wqkv_t, wv_t=wv_t, wproj_t=wproj_t, biasqk=biasqk, biasv=biasv,
        mask=mask, mscale=mscale,
    )
    in_maps = []
    for i in range(n_cores):
        sl = slice(i * tok_per_core, (i + 1) * tok_per_core)
        xa = np.ascontiguousarray(
            x_flat[sl].reshape(tok_per_core * A, DIM).T
        ).astype(bf16)
        xr = np.ascontiguousarray(
            res_flat[sl].reshape(tok_per_core * RKV, DIM).T
        ).astype(bf16)
        m = dict(shared)
        m["xt_art"] = xa
        m["xt_res"] = xr
        in_maps.append(m)
    return in_maps


def assemble_output(outs, tok_per_core=TOK_PER_CORE):
    """outs: list of (DIM, tok_per_core*8) bf16 arrays -> (B,S,A,DIM) f32."""
    parts = []
    for o in outs:
        y = np.asarray(o, dtype=np.float32)  # (1024, T*8)
        parts.append(y.reshape(DIM, tok_per_core, A).transpose(1, 2, 0))
    full = np.concatenate(parts, axis=0)  # (n_tok, A, DIM)
    if full.shape[0] == B * S:
        full = full.reshape(B, S, A, DIM)
    return np.ascontiguousarray(full)


_NC_CACHE = {}


def kernel(x, artery_embed, residual_kv, Wqkv, Wproj, mixer_scale):
    from concourse.bass_utils import run_bass_kernel_spmd

    key = TOK_PER_CORE
    if key not in _NC_CACHE:
        _NC_CACHE[key] = build_program(TOK_PER_CORE)
    nc = _NC_CACHE[key]

    in_maps = host_prep(x, artery_embed, residual_kv, Wqkv, Wproj, mixer_scale)
    res = run_bass_kernel_spmd(nc, in_maps, core_ids=list(range(N_CORES)))
    outs = [r["out_t"] for r in res.results]
    return assemble_output(outs)



# revision 42
# speedup vs baseline: 19.5902x; 19.5902x over previous
"""ArteryMixer Trainium2 kernel v3: 8-core data-parallel, fp8 DoubleRow GEMMs.

Per-token math (B=2,S=2048,A=8,R=8,DIM=1024,H=8,HD=128,SC=16):
  qkv = concat(x+emb, res) @ Wqkv.T ; q,k rmsnorm ; k_res roped (folded into W);
  scores=elu(q@k.T/sqrt(HD)) ; mixed = scores@v/16 ; out = x + scale*(mixed@Wproj.T)

Strategy (per core, 512 tokens, 8 blocks of 64):
  - artery_embed folded into the fp8 GEMM operand on host (xe8 = fp8(x+emb));
    residual path keeps bf16 x separately -> no bias adds on device.
  - Q/K/V GEMMs in fp8e4m3 with perf_mode=DoubleRow (2 contraction rows/cell);
    weights scaled x16 on host to avoid fp8 subnormals (descale folded into
    rmsnorm scale exactly and into mscale for the v path).
  - proj GEMM bf16 (accuracy headroom); attention matmuls bf16 (FD=128:
    DoubleRow loses below FD~128).
  - ACT uses only {square, exp, relu, copy} - all in the 'exp_and_others'
    activation table set, so ZERO act-table reloads. The rsqrt of rmsnorm runs
    on GPSIMD as (ssq*scale)^-0.5 via tensor_scalar pow (eps dropped: it is
    ~6e-8 relative to ssq). Norm chain: ACT square -> gp all-reduce ->
    gp pow -> gp apply-mul.
  - Ops paired into [128,1024] tiles to amortize per-instruction overhead:
    12 qkv pair-items (8 DoubleRow matmuls + 1 DVE copy each), 8 v items
    (8 DR matmuls + 1 gp copy), 12 norm pair-items, 4 proj pair-items
    (16 matmuls + 2 DVE stt + 1 DMA out).
"""

import numpy as np
import ml_dtypes

bf16 = ml_dtypes.bfloat16
f8e4 = ml_dtypes.float8_e4m3fn

HEADS = 8
HD = 128
DIM = 1024
MD = 1024
A = 8
RKV = 8
SC = 16
EPS = 1.1920929e-07
ROPE_BASE = 10000.0
N_CORES = 8
B, S = 2, 2048
TOK_PER_CORE = (B * S) // N_CORES  # 512
BLK_TOK = 64                        # tokens per pipeline block
NB = TOK_PER_CORE // BLK_TOK        # 8 blocks
CPB = BLK_TOK * 8                   # 512 cols per block (token-major, slot-minor)
WSCALE = 16.0                       # host-side fp8 weight scale


def _rope_matrix():
    inv_freq = 1.0 / (ROPE_BASE ** (np.arange(0, HD, 2, dtype=np.float64) / HD))
    c, s = np.cos(inv_freq), np.sin(inv_freq)
    Rm = np.zeros((HD, HD), dtype=np.float64)
    i = np.arange(HD // 2)
    # reference _rope: out1 = x1*c + x2*s ; out2 = -x1*s + x2*c
    Rm[i, i] = c
    Rm[i, i + 64] = s
    Rm[i + 64, i] = -s
    Rm[i + 64, i + 64] = c
    return Rm


def build_program(tok_per_core=TOK_PER_CORE, repeat=1):
    import concourse.bass as bass  # noqa
    import concourse.mybir as mybir
    import concourse.tile as tile
    from concourse import bacc
    from concourse import bass_isa

    dt = mybir.dt
    Alu = mybir.AluOpType
    Act = mybir.ActivationFunctionType
    DR = mybir.MatmulPerfMode.DoubleRow

    nb = tok_per_core // BLK_TOK
    COLS = tok_per_core * 8

    nc = bacc.Bacc(None, target_bir_lowering=False)

    xe8_t = nc.dram_tensor("xe8_t", [DIM, COLS], dt.float8e4, kind="ExternalInput")
    xr8_t = nc.dram_tensor("xr8_t", [DIM, COLS], dt.float8e4, kind="ExternalInput")
    xab_t = nc.dram_tensor("xab_t", [DIM, COLS], dt.bfloat16, kind="ExternalInput")
    wqkv_t = nc.dram_tensor("wqkv_t", [DIM, 3 * MD], dt.float8e4, kind="ExternalInput")
    wv_t = nc.dram_tensor("wv_t", [DIM, MD], dt.float8e4, kind="ExternalInput")
    wproj_t = nc.dram_tensor("wproj_t", [MD, DIM], dt.float8e4, kind="ExternalInput")
    mask_d = nc.dram_tensor("mask", [128, 128], dt.bfloat16, kind="ExternalInput")
    mscale_d = nc.dram_tensor("mscale", [128, 8], dt.float32, kind="ExternalInput")
    out_t = nc.dram_tensor("out_t", [DIM, COLS], dt.bfloat16, kind="ExternalOutput")

    with tile.TileContext(nc) as tc:
        with (
            tc.tile_pool(name="w", bufs=1) as wpool,
            tc.tile_pool(name="x", bufs=2) as xpool,
            tc.tile_pool(name="slab", bufs=2) as spool,
            tc.tile_pool(name="vslab", bufs=1) as vpool,
            tc.tile_pool(name="nrm", bufs=2) as npool,
            tc.tile_pool(name="att", bufs=2) as fpool,
            tc.tile_pool(name="rtp", bufs=3) as rtpool,
            tc.tile_pool(name="y", bufs=2) as ypool,
            tc.tile_pool(name="mm", bufs=2, space="PSUM") as mmpool,
            # scores and mixed share one double-buffered psum slot
            # (8-bank budget: mm 2x2 + scmx 2x2).
            tc.tile_pool(name="sc", bufs=2, space="PSUM") as scpool,
        ):
            # Preload the combined ln+exp activation table set: every ACT
            # function used below (Ln, Exp, Copy, Square) lives in set 6
            # ('natural_log_exp_and_others'), so no further table reloads are
            # ever needed. Without this the table pass alternates between the
            # ln-only and exp-only sets at ~1.3us per reload.
            nc.scalar.add_instruction(mybir.InstLoadActFuncSet(
                name=nc.get_next_instruction_name(), act_func_set_id=6,
                ins=[], outs=[]))

            # ---- resident weights/constants ----
            wqkv_sb = wpool.tile([128, 8, 3 * MD], dt.float8e4)
            nc.sync.dma_start(
                wqkv_sb, wqkv_t[:].rearrange("(dc p) f -> p dc f", p=128)
            )
            wv_sb = wpool.tile([128, 8, MD], dt.float8e4)
            nc.sync.dma_start(wv_sb, wv_t[:].rearrange("(dc p) f -> p dc f", p=128))
            wproj_sb = wpool.tile([128, 8, DIM], dt.float8e4)
            nc.sync.dma_start(
                wproj_sb, wproj_t[:].rearrange("(mc p) f -> p mc f", p=128)
            )
            mask_sb = wpool.tile([128, 128], dt.bfloat16)
            nc.sync.dma_start(mask_sb, mask_d[:])
            mscale_sb = wpool.tile([128, 8], dt.float32)
            nc.sync.dma_start(mscale_sb, mscale_d[:])

            xe_dram = xe8_t[:].rearrange("(dc p) c -> p dc c", p=128)
            xr_dram = xr8_t[:].rearrange("(dc p) c -> p dc c", p=128)
            xa_dram = xab_t[:].rearrange("(dc p) c -> p dc c", p=128)
            yo_dram = out_t[:].rearrange("(dc p) c -> p dc c", p=128)

            def build_gemm_items(blk):
                """Allocate block tiles + return GEMM work-item closures."""
                c0 = blk * CPB
                xe8 = xpool.tile([128, 8, CPB], dt.float8e4, tag="xe8", name="xe8")
                xr8 = xpool.tile([128, 8, CPB], dt.float8e4, tag="xr8", name="xr8")
                xab = xpool.tile([128, 8, CPB], dt.bfloat16, tag="xab", name="xab")
                nc.sync.dma_start(xe8, xe_dram[:, :, c0 : c0 + CPB])
                nc.sync.dma_start(xr8, xr_dram[:, :, c0 : c0 + CPB])
                nc.sync.dma_start(xab, xa_dram[:, :, c0 : c0 + CPB])
                qT = spool.tile([128, 8, CPB], dt.bfloat16, tag="qT")
                kTa = spool.tile([128, 8, CPB], dt.bfloat16, tag="kTa")
                kTr = spool.tile([128, 8, CPB], dt.bfloat16, tag="kTr")
                va = vpool.tile([128, 4, 8, HD], dt.bfloat16, tag="va")
                vr = vpool.tile([128, 4, 8, HD], dt.bfloat16, tag="vr")
                st = dict(xe8=xe8, xr8=xr8, xab=xab, qT=qT, kTa=kTa, kTr=kTr,
                          va=va, vr=vr, c0=c0)
                slabs = [qT, kTa, kTr]
                items = []

                def qkv_item(fp):
                    # computes fc = 2*fp, 2*fp+1 (same slab)
                    def go():
                        ps = mmpool.tile([128, 2, 512], dt.float32, tag="mmps")
                        for half in range(2):
                            fc = 2 * fp + half
                            src = xr8 if fc >= 16 else xe8
                            for j in range(4):
                                nc.tensor.matmul(
                                    ps[:, half, :],
                                    wqkv_sb[:, 2 * j : 2 * j + 2,
                                            fc * 128 : (fc + 1) * 128],
                                    src[:, 2 * j : 2 * j + 2, :],
                                    start=(j == 0),
                                    stop=(j == 3),
                                    perf_mode=DR,
                                )
                        fc0 = 2 * fp
                        dst = slabs[fc0 // 8][:, fc0 % 8 : fc0 % 8 + 2, :]
                        nc.vector.tensor_copy(dst, ps)
                    return go

                def v_item(isart, rc):
                    alt = (rc + (0 if isart else 1)) % 2
                    def go():
                        src, dstv = (xe8, va) if isart else (xr8, vr)
                        ps = mmpool.tile([128, 2, 512], dt.float32, tag="mmps")
                        for vh in range(2):
                            for j in range(4):
                                nc.tensor.matmul(
                                    ps[:, vh, :],
                                    src[:, 2 * j : 2 * j + 2,
                                        rc * 128 : (rc + 1) * 128],
                                    wv_sb[:, 2 * j : 2 * j + 2,
                                          vh * 512 : (vh + 1) * 512],
                                    start=(j == 0),
                                    stop=(j == 3),
                                    perf_mode=DR,
                                )
                        dv = dstv[:, rc, :, :]
                        # psum readers must be ACT or DVE (GPSIMD cannot
                        # access PSUM on real hardware).
                        if alt == 0:
                            nc.scalar.copy(out=dv, in_=ps)
                        else:
                            nc.vector.tensor_copy(dv, ps)
                    return go

                for fp in range(12):
                    items.append(qkv_item(fp))
                for isart in (True, False):
                    for rc in range(4):
                        items.append(v_item(isart, rc))
                return st, items

            def build_attn_items(st):
                """Work items for norm + attention + proj of a block."""
                qT, kTa, kTr = st["qT"], st["kTa"], st["kTr"]
                va, vr, xab, c0 = st["va"], st["vr"], st["xab"], st["c0"]

                def norm_item(slab, scv, hp):
                    # normalizes head-chunks 2*hp, 2*hp+1 of slab:
                    # gp square -> gp all-reduce -> rsqrt as exp(-0.5*ln(m))
                    # on ACT (ln and exp share ONE activation table set, so no
                    # table reloads; Sqrt/Rsqrt/Reciprocal would all swap
                    # tables against the elu exp) -> gp apply.
                    def go():
                        with nc.allow_low_precision(
                            reason="all-reduce upcasts internally; bf16 ~0.4%"
                        ):
                            sl = slab[:, 2 * hp : 2 * hp + 2, :]
                            sq = npool.tile([128, 2, CPB], dt.bfloat16, tag="nsq")
                            nc.gpsimd.tensor_mul(sq, sl, sl)
                            ssq = npool.tile([128, 2, CPB], dt.bfloat16, tag="nssq")
                            for hf in range(2):
                                nc.gpsimd.partition_all_reduce(
                                    ssq[:, hf, :], sq[:, hf, :], channels=128,
                                    reduce_op=bass_isa.ReduceOp.add,
                                )
                            lnm = npool.tile([128, 2, CPB], dt.bfloat16, tag="nsq")
                            nc.scalar.activation(lnm, ssq, Act.Ln, scale=scv)
                            rs = npool.tile([128, 2, CPB], dt.bfloat16, tag="nssq")
                            nc.scalar.activation(rs, lnm, Act.Exp, scale=-0.5)
                            nc.gpsimd.tensor_mul(sl, sl, rs)
                    return go

                mixedT = vpool.tile([128, 8, CPB], dt.float8e4, tag="mixedT")
                st["mixedT"] = mixedT
                routes = {}

                def scores_item(g, half):
                    def go():
                        gsl = slice(g * 128, (g + 1) * 128)
                        kT = kTa if half == 0 else kTr
                        ps = scpool.tile([128, 8, 128], dt.float32, tag="scps")
                        for h in range(8):
                            nc.tensor.matmul(
                                ps[:, h, :], kT[:, h, gsl], qT[:, h, gsl],
                                start=True, stop=True,
                            )
                        esc = fpool.tile([128, 8, 128], dt.bfloat16, tag="ers")
                        rsc = fpool.tile([128, 8, 128], dt.bfloat16, tag="ers")
                        nc.scalar.activation(esc, ps, Act.Exp)
                        # elu = relu(s) + (min(exp(s),1) - 1); relu comes from
                        # the psum via DVE stt, saving an ACT op.
                        nc.vector.tensor_scalar(esc, esc, 1.0, -1.0, Alu.min, Alu.add)
                        nc.vector.scalar_tensor_tensor(
                            out=rsc, in0=ps, scalar=0.0, in1=esc,
                            op0=Alu.max, op1=Alu.add,
                        )
                        route = rtpool.tile([128, 8, 128], dt.bfloat16, tag="rt")
                        nc.gpsimd.tensor_mul(
                            route, rsc,
                            mask_sb[:, None, :].to_broadcast((128, 8, 128)),
                        )
                        routes[(g, half)] = route
                    return go

                def mixed_item(g):
                    def go():
                        gsl = slice(g * 128, (g + 1) * 128)
                        mx = scpool.tile([128, 8, 128], dt.float32, tag="scps")
                        for h in range(8):
                            nc.tensor.matmul(
                                mx[:, h, :], va[:, g, h, :],
                                routes[(g, 0)][:, h, :], start=True, stop=False,
                            )
                            nc.tensor.matmul(
                                mx[:, h, :], vr[:, g, h, :],
                                routes[(g, 1)][:, h, :], start=False, stop=True,
                            )
                        if g % 2 == 0:
                            nc.scalar.copy(out=mixedT[:, :, gsl], in_=mx)
                        else:
                            nc.vector.tensor_copy(mixedT[:, :, gsl], mx)
                    return go

                def proj_item(dp):
                    # projects feature chunks dc = 2*dp, 2*dp+1
                    def go():
                        ps = mmpool.tile([128, 2, 512], dt.float32, tag="mmps")
                        yb = ypool.tile([128, 2, CPB], dt.bfloat16, tag="yb")
                        for half in range(2):
                            dc = 2 * dp + half
                            for j in range(4):
                                nc.tensor.matmul(
                                    ps[:, half, :],
                                    wproj_sb[:, 2 * j : 2 * j + 2,
                                             dc * 128 : (dc + 1) * 128],
                                    mixedT[:, 2 * j : 2 * j + 2, :],
                                    start=(j == 0), stop=(j == 3),
                                    perf_mode=DR,
                                )
                            nc.vector.scalar_tensor_tensor(
                                out=yb[:, half, :], in0=ps[:, half, :],
                                scalar=mscale_sb[:, dc : dc + 1],
                                in1=xab[:, dc, :],
                                op0=Alu.mult, op1=Alu.add,
                            )
                        nc.sync.dma_start(
                            yo_dram[:, 2 * dp : 2 * dp + 2, c0 : c0 + CPB], yb
                        )
                    return go

                norm_items = []
                for slab, scv in (
                    (qT, 1.0 / HD),
                    (kTa, 1.0),
                    (kTr, 1.0),
                ):
                    for hp in range(4):
                        norm_items.append(norm_item(slab, scv, hp))
                return dict(
                    norm=norm_items,
                    groups=[(scores_item(g, 0), scores_item(g, 1), mixed_item(g))
                            for g in range(4)],
                    proj=[proj_item(dp) for dp in range(4)],
                )

            def merge(attn, gemm):
                """Structured interleave: norm 3:1 with gemms, then per group
                sc,G,sc,G,mx,G, then proj 1:1 with gemms; leftovers last."""
                out = []
                gq = list(gemm)

                def g(n):
                    for _ in range(n):
                        if gq:
                            out.append(gq.pop(0))

                if attn is None:
                    return list(gemm)
                for i, it in enumerate(attn["norm"]):
                    out.append(it)
                    if i % 3 == 2:
                        g(1)
                for sc0, sc1, mx in attn["groups"]:
                    out.append(sc0); g(1)
                    out.append(sc1); g(1)
                    out.append(mx); g(1)
                for p in attn["proj"]:
                    out.append(p); g(1)
                out.extend(gq)
                return out

            blklist = [b for _ in range(repeat) for b in range(nb)]
            prev_st = None
            for i in range(len(blklist) + 1):
                gemm_items = []
                if i < len(blklist):
                    st, gemm_items = build_gemm_items(blklist[i])
                attn = build_attn_items(prev_st) if prev_st is not None else None
                for item in merge(attn, gemm_items):
                    item()
                if i < len(blklist):
                    prev_st = st

    nc.compile()
    return nc


def host_prep(x, artery_embed, residual_kv, Wqkv, Wproj, mixer_scale,
              tok_per_core=TOK_PER_CORE, n_cores=N_CORES):
    T = x.shape[0] * x.shape[1]
    x_flat = np.asarray(x, dtype=np.float32).reshape(T, A, DIM)
    res_flat = np.asarray(residual_kv, dtype=np.float32).reshape(T, RKV, DIM)
    emb = np.asarray(artery_embed, dtype=np.float32)
    xe_flat = x_flat + emb[None]

    Rm = _rope_matrix()
    Wq = np.asarray(Wqkv[0:MD], dtype=np.float64)
    Wk = np.asarray(Wqkv[MD : 2 * MD], dtype=np.float64)
    Wv = np.asarray(Wqkv[2 * MD : 3 * MD], dtype=np.float64)
    Wk_res = np.einsum("de,hec->hdc", Rm, Wk.reshape(HEADS, HD, DIM)).reshape(MD, DIM)

    wqkv_t = np.ascontiguousarray(
        np.concatenate([Wq, Wk, Wk_res], axis=0).T * WSCALE
    ).astype(f8e4)
    wv_t = np.ascontiguousarray(Wv.T * WSCALE).astype(f8e4)
    wproj_t = np.ascontiguousarray(
        np.asarray(Wproj, dtype=np.float64).T * WSCALE
    ).astype(f8e4)

    mask = np.zeros((128, 128), dtype=np.float32)
    for t in range(16):
        mask[t * 8 : (t + 1) * 8, t * 8 : (t + 1) * 8] = 1.0 / SC
    mask = mask.astype(bf16)

    # v path carries WSCALE (through mixedT) and Wproj carries another
    # WSCALE; descale both via mscale.
    mscale = np.ascontiguousarray(
        (np.asarray(mixer_scale, dtype=np.float32) / (WSCALE * WSCALE))
        .reshape(8, 128).T
    )

    shared = dict(
        wqkv_t=wqkv_t, wv_t=wv_t, wproj_t=wproj_t, mask=mask, mscale=mscale,
    )
    in_maps = []
    for i in range(n_cores):
        sl = slice(i * tok_per_core, (i + 1) * tok_per_core)
        xe = np.ascontiguousarray(
            xe_flat[sl].reshape(tok_per_core * A, DIM).T
        ).astype(f8e4)
        xr = np.ascontiguousarray(
            res_flat[sl].reshape(tok_per_core * RKV, DIM).T
        ).astype(f8e4)
        xa = np.ascontiguousarray(
            x_flat[sl].reshape(tok_per_core * A, DIM).T
        ).astype(bf16)
        m = dict(shared)
        m["xe8_t"] = xe
        m["xr8_t"] = xr
        m["xab_t"] = xa
        in_maps.append(m)
    return in_maps


def assemble_output(outs, tok_per_core=TOK_PER_CORE):
    """outs: list of (DIM, tok_per_core*8) bf16 arrays -> (B,S,A,DIM) f32."""
    parts = []
    for o in outs:
        y = np.asarray(o, dtype=np.float32)  # (1024, T*8)
        parts.append(y.reshape(DIM, tok_per_core, A).transpose(1, 2, 0))
    full = np.concatenate(parts, axis=0)  # (n_tok, A, DIM)
    if full.shape[0] == B * S:
        full = full.reshape(B, S, A, DIM)
    return np.ascontiguousarray(full)


_NC_CACHE = {}


def kernel(x, artery_embed, residual_kv, Wqkv, Wproj, mixer_scale):
    from concourse.bass_utils import run_bass_kernel_spmd

    key = TOK_PER_CORE
    if key not in _NC_CACHE:
        _NC_CACHE[key] = build_program(TOK_PER_CORE)
    nc = _NC_CACHE[key]

    in_maps = host_prep(x, artery_embed, residual_kv, Wqkv, Wproj, mixer_scale)
    res = run_bass_kernel_spmd(nc, in_maps, core_ids=list(range(N_CORES)))
    outs = [r["out_t"] for r in res.results]
    return assemble_output(outs)


# revision 43
# speedup vs baseline: 44.1755x; 2.2550x over previous
"""ArteryMixer Trainium2 kernel v12: v1 pipeline structure + fp8 DoubleRow GEMMs.

Per-token math (B=2,S=2048,A=8,R=8,DIM=1024,H=8,HD=128,SC=16):
  qkv = concat(x+emb, res) @ Wqkv.T ; q,k rmsnorm ; k_res roped (folded into W);
  scores=elu(q@k.T/sqrt(HD)) ; mixed = scores@v/16 ; out = x + scale*(mixed@Wproj.T)

Deltas vs the HW-proven v1 schedule (which this keeps exactly):
  - artery_embed folded into the fp8 GEMM operand on host (xe8 = fp8(x+emb));
    bias adds on device are gone. Residual add keeps a separate bf16 x.
  - QKV / V / proj GEMMs in fp8e4m3 with perf_mode=DoubleRow (half the PE
    cycles); weights scaled x16 on host to dodge fp8 subnormals; descale is
    folded into the rmsnorm eps (exact) and mscale (v/proj path, /256).
  - mixedT stored fp8 (feeds the fp8 proj GEMM).
  - attention matmuls stay bf16 (FD=128 is below DoubleRow's win threshold).
"""

import numpy as np
import ml_dtypes

bf16 = ml_dtypes.bfloat16
f8e4 = ml_dtypes.float8_e4m3fn

HEADS = 8
HD = 128
DIM = 1024
MD = 1024
A = 8
RKV = 8
SC = 16
EPS = 1.1920929e-07
ROPE_BASE = 10000.0
N_CORES = 8
B, S = 2, 2048
TOK_PER_CORE = (B * S) // N_CORES  # 512
BLK_TOK = 64                        # tokens per pipeline block
NB = TOK_PER_CORE // BLK_TOK        # 8 blocks
CPB = BLK_TOK * 8                   # 512 cols per block (token-major, slot-minor)
WSCALE = 16.0                       # host-side fp8 weight scale


def _rope_matrix():
    inv_freq = 1.0 / (ROPE_BASE ** (np.arange(0, HD, 2, dtype=np.float64) / HD))
    c, s = np.cos(inv_freq), np.sin(inv_freq)
    Rm = np.zeros((HD, HD), dtype=np.float64)
    i = np.arange(HD // 2)
    # reference _rope: out1 = x1*c + x2*s ; out2 = -x1*s + x2*c
    Rm[i, i] = c
    Rm[i, i + 64] = s
    Rm[i + 64, i] = -s
    Rm[i + 64, i + 64] = c
    return Rm


def build_program(tok_per_core=TOK_PER_CORE, repeat=1):
    import concourse.bass as bass  # noqa
    import concourse.mybir as mybir
    import concourse.tile as tile
    from concourse import bacc
    from concourse import bass_isa

    dt = mybir.dt
    Alu = mybir.AluOpType
    Act = mybir.ActivationFunctionType
    DR = mybir.MatmulPerfMode.DoubleRow

    nb = tok_per_core // BLK_TOK
    COLS = tok_per_core * 8

    nc = bacc.Bacc(None, target_bir_lowering=False)

    xe8_t = nc.dram_tensor("xe8_t", [DIM, COLS], dt.float8e4, kind="ExternalInput")
    xr8_t = nc.dram_tensor("xr8_t", [DIM, COLS], dt.float8e4, kind="ExternalInput")
    xab_t = nc.dram_tensor("xab_t", [DIM, COLS], dt.bfloat16, kind="ExternalInput")
    wqkv_t = nc.dram_tensor("wqkv_t", [DIM, 3 * MD], dt.float8e4, kind="ExternalInput")
    wv_t = nc.dram_tensor("wv_t", [DIM, MD], dt.float8e4, kind="ExternalInput")
    wproj_t = nc.dram_tensor("wproj_t", [MD, DIM], dt.float8e4, kind="ExternalInput")
    mask_d = nc.dram_tensor("mask", [128, 128], dt.bfloat16, kind="ExternalInput")
    mscale_d = nc.dram_tensor("mscale", [128, 8], dt.float32, kind="ExternalInput")
    out_t = nc.dram_tensor("out_t", [DIM, COLS], dt.bfloat16, kind="ExternalOutput")

    with tile.TileContext(nc) as tc:
        with (
            tc.tile_pool(name="w", bufs=1) as wpool,
            tc.tile_pool(name="x", bufs=2) as xpool,
            tc.tile_pool(name="slab", bufs=2) as spool,
            tc.tile_pool(name="vslab", bufs=1) as vpool,
            tc.tile_pool(name="nrm", bufs=2) as npool,
            tc.tile_pool(name="att", bufs=2) as fpool,
            tc.tile_pool(name="rtp", bufs=3) as rtpool,
            tc.tile_pool(name="y", bufs=2) as ypool,
            tc.tile_pool(name="mm", bufs=2, space="PSUM") as mmpool,
            tc.tile_pool(name="sc", bufs=2, space="PSUM") as scpool,
            tc.tile_pool(name="mx", bufs=1, space="PSUM") as mxpool,
        ):
            # ---- resident weights/constants ----
            wqkv_sb = wpool.tile([128, 8, 3 * MD], dt.float8e4)
            nc.sync.dma_start(
                wqkv_sb, wqkv_t[:].rearrange("(dc p) f -> p dc f", p=128)
            )
            wv_sb = wpool.tile([128, 8, MD], dt.float8e4)
            nc.sync.dma_start(wv_sb, wv_t[:].rearrange("(dc p) f -> p dc f", p=128))
            wproj_sb = wpool.tile([128, 8, DIM], dt.float8e4)
            nc.sync.dma_start(
                wproj_sb, wproj_t[:].rearrange("(mc p) f -> p mc f", p=128)
            )
            mask_sb = wpool.tile([128, 128], dt.bfloat16)
            nc.sync.dma_start(mask_sb, mask_d[:])
            mscale_sb = wpool.tile([128, 8], dt.float32)
            nc.sync.dma_start(mscale_sb, mscale_d[:])
            # weights carry WSCALE -> ssq carries WSCALE^2; eps scaled to match
            # keeps the rmsnorm exactly equal to the reference's.
            eps_q = wpool.tile([128, 1], dt.float32)
            nc.vector.memset(eps_q, WSCALE * WSCALE * EPS)
            eps_k = wpool.tile([128, 1], dt.float32)
            nc.vector.memset(eps_k, WSCALE * WSCALE * HD * EPS)

            xe_dram = xe8_t[:].rearrange("(dc p) c -> p dc c", p=128)
            xr_dram = xr8_t[:].rearrange("(dc p) c -> p dc c", p=128)
            xa_dram = xab_t[:].rearrange("(dc p) c -> p dc c", p=128)
            yo_dram = out_t[:].rearrange("(dc p) c -> p dc c", p=128)

            def build_gemm_items(blk):
                """Allocate block tiles + return GEMM work-item closures."""
                c0 = blk * CPB
                xe8 = xpool.tile([128, 8, CPB], dt.float8e4, tag="xe8", name="xe8")
                xr8 = xpool.tile([128, 8, CPB], dt.float8e4, tag="xr8", name="xr8")
                xab = xpool.tile([128, 8, CPB], dt.bfloat16, tag="xab", name="xab")
                nc.sync.dma_start(xe8, xe_dram[:, :, c0 : c0 + CPB])
                nc.sync.dma_start(xr8, xr_dram[:, :, c0 : c0 + CPB])
                nc.sync.dma_start(xab, xa_dram[:, :, c0 : c0 + CPB])
                qT = spool.tile([128, 8, CPB], dt.bfloat16, tag="qT")
                kTa = spool.tile([128, 8, CPB], dt.bfloat16, tag="kTa")
                kTr = spool.tile([128, 8, CPB], dt.bfloat16, tag="kTr")
                va = vpool.tile([128, 4, 8, HD], dt.bfloat16, tag="va")
                vr = vpool.tile([128, 4, 8, HD], dt.bfloat16, tag="vr")
                st = dict(xe8=xe8, xr8=xr8, xab=xab, qT=qT, kTa=kTa, kTr=kTr,
                          va=va, vr=vr, c0=c0)
                slabs = [qT, kTa, kTr]
                items = []

                def qkv_item(fc):
                    def go():
                        ps = mmpool.tile([128, CPB], dt.float32, tag="mmps")
                        src = xr8 if fc >= 16 else xe8
                        for j in range(4):
                            nc.tensor.matmul(
                                ps,
                                wqkv_sb[:, 2 * j : 2 * j + 2,
                                        fc * 128 : (fc + 1) * 128],
                                src[:, 2 * j : 2 * j + 2, :],
                                start=(j == 0),
                                stop=(j == 3),
                                perf_mode=DR,
                            )
                        dst = slabs[fc // 8][:, fc % 8, :]
                        nc.scalar.copy(out=dst, in_=ps)
                    return go

                def v_item(isart, rc, vh):
                    def go():
                        src, dstv = (xe8, va) if isart else (xr8, vr)
                        ps = mmpool.tile([128, 512], dt.float32, tag="mmps")
                        for j in range(4):
                            nc.tensor.matmul(
                                ps,
                                src[:, 2 * j : 2 * j + 2, rc * 128 : (rc + 1) * 128],
                                wv_sb[:, 2 * j : 2 * j + 2,
                                      vh * 512 : (vh + 1) * 512],
                                start=(j == 0),
                                stop=(j == 3),
                                perf_mode=DR,
                            )
                        dv = dstv[:, rc, vh * 4 : (vh + 1) * 4, :]
                        nc.scalar.copy(out=dv, in_=ps)
                    return go

                for fc in range(24):
                    items.append(qkv_item(fc))
                for isart in (True, False):
                    for rc in range(4):
                        for vh in range(2):
                            items.append(v_item(isart, rc, vh))
                return st, items

            def build_attn_items(st):
                """Work items for norm + attention + proj of a block."""
                qT, kTa, kTr = st["qT"], st["kTa"], st["kTr"]
                va, vr, xab, c0 = st["va"], st["vr"], st["xab"], st["c0"]
                items = []

                def norm_item(slab, epsv, scv, hh):
                    def go():
                        with nc.allow_low_precision(
                            reason="all-reduce upcasts internally; bf16 ~0.4%"
                        ):
                            sl = slab[:, hh, :]
                            sq = npool.tile([128, CPB], dt.bfloat16, tag="nsq")
                            nc.vector.tensor_mul(sq, sl, sl)
                            ssq = npool.tile([128, CPB], dt.bfloat16, tag="nssq")
                            nc.gpsimd.partition_all_reduce(
                                ssq, sq, channels=128,
                                reduce_op=bass_isa.ReduceOp.add,
                            )
                            rt = npool.tile([128, CPB], dt.bfloat16, tag="nsq")
                            nc.scalar.activation(
                                rt, ssq, Act.Sqrt, bias=epsv, scale=scv
                            )
                            rs = npool.tile([128, CPB], dt.bfloat16, tag="nssq")
                            nc.vector.reciprocal(rs, rt)
                            nc.vector.tensor_mul(sl, sl, rs)
                    return go

                mixedT = vpool.tile([128, 8, CPB], dt.float8e4, tag="mixedT")
                st["mixedT"] = mixedT
                routes = {}

                def scores_item(g, half):
                    def go():
                        gsl = slice(g * 128, (g + 1) * 128)
                        kT = kTa if half == 0 else kTr
                        ps = scpool.tile([128, 8, 128], dt.float32, tag="scps")
                        for h in range(8):
                            nc.tensor.matmul(
                                ps[:, h, :], kT[:, h, gsl], qT[:, h, gsl],
                                start=True, stop=True,
                            )
                        esc = fpool.tile([128, 8, 128], dt.bfloat16, tag="ers")
                        rsc = fpool.tile([128, 8, 128], dt.bfloat16, tag="ers")
                        nc.scalar.activation(esc, ps, Act.Exp)
                        nc.scalar.activation(rsc, ps, Act.Relu)
                        # elu = relu(s) + (min(exp(s),1) - 1)
                        nc.vector.tensor_scalar(esc, esc, 1.0, -1.0, Alu.min, Alu.add)
                        nc.vector.tensor_add(esc, rsc, esc)
                        route = rtpool.tile([128, 8, 128], dt.bfloat16, tag="rt")
                        nc.vector.tensor_mul(
                            route, esc,
                            mask_sb[:, None, :].to_broadcast((128, 8, 128)),
                        )
                        routes[(g, half)] = route
                    return go

                def mixed_item(g):
                    def go():
                        gsl = slice(g * 128, (g + 1) * 128)
                        mx = mxpool.tile([128, 8, 128], dt.float32, tag="mxps")
                        for h in range(8):
                            nc.tensor.matmul(
                                mx[:, h, :], va[:, g, h, :],
                                routes[(g, 0)][:, h, :], start=True, stop=False,
                            )
                            nc.tensor.matmul(
                                mx[:, h, :], vr[:, g, h, :],
                                routes[(g, 1)][:, h, :], start=False, stop=True,
                            )
                        nc.scalar.copy(out=mixedT[:, :, gsl], in_=mx)
                    return go

                def proj_item(dc):
                    def go():
                        ps = mmpool.tile([128, CPB], dt.float32, tag="mmps")
                        for j in range(4):
                            nc.tensor.matmul(
                                ps,
                                wproj_sb[:, 2 * j : 2 * j + 2,
                                         dc * 128 : (dc + 1) * 128],
                                mixedT[:, 2 * j : 2 * j + 2, :],
                                start=(j == 0), stop=(j == 3),
                                perf_mode=DR,
                            )
                        yb = ypool.tile([128, CPB], dt.bfloat16, tag="yb")
                        nc.vector.scalar_tensor_tensor(
                            out=yb, in0=ps, scalar=mscale_sb[:, dc : dc + 1],
                            in1=xab[:, dc, :],
                            op0=Alu.mult, op1=Alu.add,
                        )
                        nc.sync.dma_start(yo_dram[:, dc, c0 : c0 + CPB], yb)
                    return go

                norm_items = []
                for slab, epsv, scv in (
                    (qT, eps_q, 1.0 / HD),
                    (kTa, eps_k, 1.0),
                    (kTr, eps_k, 1.0),
                ):
                    for hh in range(8):
                        norm_items.append(norm_item(slab, epsv, scv, hh))
                return dict(
                    norm=norm_items,
                    groups=[(scores_item(g, 0), scores_item(g, 1), mixed_item(g))
                            for g in range(4)],
                    proj=[proj_item(dc) for dc in range(8)],
                )

            def merge(attn, gemm):
                """Structured interleave: norm 3:1 with gemms, then per group
                sc,G,sc,G,G,mx,G, then proj 1:2 with gemms; leftovers last."""
                out = []
                gq = list(gemm)

                def g(n):
                    for _ in range(n):
                        if gq:
                            out.append(gq.pop(0))

                if attn is None:
                    return list(gemm)
                for i, it in enumerate(attn["norm"]):
                    out.append(it)
                    if i % 3 == 2:
                        g(1)
                for sc0, sc1, mx in attn["groups"]:
                    out.append(sc0); g(1)
                    out.append(sc1); g(2)
                    out.append(mx); g(1)
                for p in attn["proj"]:
                    out.append(p); g(2)
                out.extend(gq)
                return out

            blklist = [b for _ in range(repeat) for b in range(nb)]
            prev_st = None
            for i in range(len(blklist) + 1):
                gemm_items = []
                if i < len(blklist):
                    st, gemm_items = build_gemm_items(blklist[i])
                attn = build_attn_items(prev_st) if prev_st is not None else None
                for item in merge(attn, gemm_items):
                    item()
                if i < len(blklist):
                    prev_st = st

    nc.compile()
    return nc


def host_prep(x, artery_embed, residual_kv, Wqkv, Wproj, mixer_scale,
              tok_per_core=TOK_PER_CORE, n_cores=N_CORES):
    T = x.shape[0] * x.shape[1]
    x_flat = np.asarray(x, dtype=np.float32).reshape(T, A, DIM)
    res_flat = np.asarray(residual_kv, dtype=np.float32).reshape(T, RKV, DIM)
    emb = np.asarray(artery_embed, dtype=np.float32)
    xe_flat = x_flat + emb[None]

    Rm = _rope_matrix()
    Wq = np.asarray(Wqkv[0:MD], dtype=np.float64)
    Wk = np.asarray(Wqkv[MD : 2 * MD], dtype=np.float64)
    Wv = np.asarray(Wqkv[2 * MD : 3 * MD], dtype=np.float64)
    Wk_res = np.einsum("de,hec->hdc", Rm, Wk.reshape(HEADS, HD, DIM)).reshape(MD, DIM)

    wqkv_t = np.ascontiguousarray(
        np.concatenate([Wq, Wk, Wk_res], axis=0).T * WSCALE
    ).astype(f8e4)
    wv_t = np.ascontiguousarray(Wv.T * WSCALE).astype(f8e4)
    wproj_t = np.ascontiguousarray(
        np.asarray(Wproj, dtype=np.float64).T * WSCALE
    ).astype(f8e4)

    mask = np.zeros((128, 128), dtype=np.float32)
    for t in range(16):
        mask[t * 8 : (t + 1) * 8, t * 8 : (t + 1) * 8] = 1.0 / SC
    mask = mask.astype(bf16)

    # v path and proj each carry WSCALE; descale both via mscale.
    mscale = np.ascontiguousarray(
        (np.asarray(mixer_scale, dtype=np.float32) / (WSCALE * WSCALE))
        .reshape(8, 128).T
    )

    shared = dict(
        wqkv_t=wqkv_t, wv_t=wv_t, wproj_t=wproj_t, mask=mask, mscale=mscale,
    )
    in_maps = []
    for i in range(n_cores):
        sl = slice(i * tok_per_core, (i + 1) * tok_per_core)
        xe = np.ascontiguousarray(
            xe_flat[sl].reshape(tok_per_core * A, DIM).T
        ).astype(f8e4)
        xr = np.ascontiguousarray(
            res_flat[sl].reshape(tok_per_core * RKV, DIM).T
        ).astype(f8e4)
        xa = np.ascontiguousarray(
            x_flat[sl].reshape(tok_per_core * A, DIM).T
        ).astype(bf16)
        m = dict(shared)
        m["xe8_t"] = xe
        m["xr8_t"] = xr
        m["xab_t"] = xa
        in_maps.append(m)
    return in_maps


def assemble_output(outs, tok_per_core=TOK_PER_CORE):
    """outs: list of (DIM, tok_per_core*8) bf16 arrays -> (B,S,A,DIM) f32."""
    parts = []
    for o in outs:
        y = np.asarray(o, dtype=np.float32)  # (1024, T*8)
        parts.append(y.reshape(DIM, tok_per_core, A).transpose(1, 2, 0))
    full = np.concatenate(parts, axis=0)  # (n_tok, A, DIM)
    if full.shape[0] == B * S:
        full = full.reshape(B, S, A, DIM)
    return np.ascontiguousarray(full)


_NC_CACHE = {}


def kernel(x, artery_embed, residual_kv, Wqkv, Wproj, mixer_scale):
    from concourse.bass_utils import run_bass_kernel_spmd

    key = TOK_PER_CORE
    if key not in _NC_CACHE:
        _NC_CACHE[key] = build_program(TOK_PER_CORE)
    nc = _NC_CACHE[key]

    in_maps = host_prep(x, artery_embed, residual_kv, Wqkv, Wproj, mixer_scale)
    res = run_bass_kernel_spmd(nc, in_maps, core_ids=list(range(N_CORES)))
    outs = [r["out_t"] for r in res.results]
    return assemble_output(outs)


# revision 49
# speedup vs baseline: 53.0681x; 1.2013x over previous
"""ArteryMixer Trainium2 kernel v12: v1 pipeline structure + fp8 DoubleRow GEMMs.

Per-token math (B=2,S=2048,A=8,R=8,DIM=1024,H=8,HD=128,SC=16):
  qkv = concat(x+emb, res) @ Wqkv.T ; q,k rmsnorm ; k_res roped (folded into W);
  scores=elu(q@k.T/sqrt(HD)) ; mixed = scores@v/16 ; out = x + scale*(mixed@Wproj.T)

Deltas vs the HW-proven v1 schedule (which this keeps exactly):
  - artery_embed folded into the fp8 GEMM operand on host (xe8 = fp8(x+emb));
    bias adds on device are gone. Residual add keeps a separate bf16 x.
  - QKV / V / proj GEMMs in fp8e4m3 with perf_mode=DoubleRow (half the PE
    cycles); weights scaled x16 on host to dodge fp8 subnormals; descale is
    folded into the rmsnorm eps (exact) and mscale (v/proj path, /256).
  - mixedT stored fp8 (feeds the fp8 proj GEMM).
  - attention matmuls stay bf16 (FD=128 is below DoubleRow's win threshold).
"""

import numpy as np
import ml_dtypes

bf16 = ml_dtypes.bfloat16
f8e4 = ml_dtypes.float8_e4m3fn

HEADS = 8
HD = 128
DIM = 1024
MD = 1024
A = 8
RKV = 8
SC = 16
EPS = 1.1920929e-07
ROPE_BASE = 10000.0
N_CORES = 8
B, S = 2, 2048
TOK_PER_CORE = (B * S) // N_CORES  # 512
BLK_TOK = 64                        # tokens per pipeline block
NB = TOK_PER_CORE // BLK_TOK        # 8 blocks
CPB = BLK_TOK * 8                   # 512 cols per block (token-major, slot-minor)
WSCALE = 16.0                       # host-side fp8 weight scale


def _rope_matrix():
    inv_freq = 1.0 / (ROPE_BASE ** (np.arange(0, HD, 2, dtype=np.float64) / HD))
    c, s = np.cos(inv_freq), np.sin(inv_freq)
    Rm = np.zeros((HD, HD), dtype=np.float64)
    i = np.arange(HD // 2)
    # reference _rope: out1 = x1*c + x2*s ; out2 = -x1*s + x2*c
    Rm[i, i] = c
    Rm[i, i + 64] = s
    Rm[i + 64, i] = -s
    Rm[i + 64, i + 64] = c
    return Rm


def build_program(tok_per_core=TOK_PER_CORE, repeat=1):
    import concourse.bass as bass  # noqa
    import concourse.mybir as mybir
    import concourse.tile as tile
    from concourse import bacc
    from concourse import bass_isa

    dt = mybir.dt
    Alu = mybir.AluOpType
    Act = mybir.ActivationFunctionType
    DR = mybir.MatmulPerfMode.DoubleRow

    nb = tok_per_core // BLK_TOK
    COLS = tok_per_core * 8

    nc = bacc.Bacc(None, target_bir_lowering=False)

    xe8_t = nc.dram_tensor("xe8_t", [DIM, COLS], dt.float8e4, kind="ExternalInput")
    xr8_t = nc.dram_tensor("xr8_t", [DIM, COLS], dt.float8e4, kind="ExternalInput")
    xab_t = nc.dram_tensor("xab_t", [DIM, COLS], dt.bfloat16, kind="ExternalInput")
    wqkv_t = nc.dram_tensor("wqkv_t", [DIM, 3 * MD], dt.float8e4, kind="ExternalInput")
    wv_t = nc.dram_tensor("wv_t", [DIM, MD], dt.float8e4, kind="ExternalInput")
    wproj_t = nc.dram_tensor("wproj_t", [MD, DIM], dt.float8e4, kind="ExternalInput")
    mask_d = nc.dram_tensor("mask", [128, 128], dt.bfloat16, kind="ExternalInput")
    mscale_d = nc.dram_tensor("mscale", [128, 8], dt.float32, kind="ExternalInput")
    out_t = nc.dram_tensor("out_t", [DIM, COLS], dt.bfloat16, kind="ExternalOutput")

    with tile.TileContext(nc) as tc:
        with (
            tc.tile_pool(name="w", bufs=1) as wpool,
            tc.tile_pool(name="x", bufs=2) as xpool,
            tc.tile_pool(name="slab", bufs=2) as spool,
            tc.tile_pool(name="vslab", bufs=1) as vpool,
            tc.tile_pool(name="nrm", bufs=2) as npool,
            tc.tile_pool(name="att", bufs=2) as fpool,
            tc.tile_pool(name="rtp", bufs=3) as rtpool,
            tc.tile_pool(name="y", bufs=2) as ypool,
            tc.tile_pool(name="mm", bufs=2, space="PSUM") as mmpool,
            tc.tile_pool(name="sc", bufs=2, space="PSUM") as scpool,
        ):
            # Preload the combined ln+exp activation table set (set 6,
            # 'natural_log_exp_and_others'): every ACT function used below
            # (Ln, Exp, Relu, Copy) lives in it, so no table reloads ever.
            nc.scalar.add_instruction(mybir.InstLoadActFuncSet(
                name=nc.get_next_instruction_name(), act_func_set_id=6,
                ins=[], outs=[]))

            # ---- resident weights/constants ----
            wqkv_sb = wpool.tile([128, 8, 3 * MD], dt.float8e4)
            nc.sync.dma_start(
                wqkv_sb, wqkv_t[:].rearrange("(dc p) f -> p dc f", p=128)
            )
            wv_sb = wpool.tile([128, 8, MD], dt.float8e4)
            nc.sync.dma_start(wv_sb, wv_t[:].rearrange("(dc p) f -> p dc f", p=128))
            wproj_sb = wpool.tile([128, 8, DIM], dt.float8e4)
            nc.sync.dma_start(
                wproj_sb, wproj_t[:].rearrange("(mc p) f -> p mc f", p=128)
            )
            mask_sb = wpool.tile([128, 128], dt.bfloat16)
            nc.sync.dma_start(mask_sb, mask_d[:])
            mscale_sb = wpool.tile([128, 8], dt.float32)
            nc.sync.dma_start(mscale_sb, mscale_d[:])

            xe_dram = xe8_t[:].rearrange("(dc p) c -> p dc c", p=128)
            xr_dram = xr8_t[:].rearrange("(dc p) c -> p dc c", p=128)
            xa_dram = xab_t[:].rearrange("(dc p) c -> p dc c", p=128)
            yo_dram = out_t[:].rearrange("(dc p) c -> p dc c", p=128)

            def build_gemm_items(blk):
                """Allocate block tiles + return GEMM work-item closures."""
                c0 = blk * CPB
                xe8 = xpool.tile([128, 8, CPB], dt.float8e4, tag="xe8", name="xe8")
                xr8 = xpool.tile([128, 8, CPB], dt.float8e4, tag="xr8", name="xr8")
                xab = xpool.tile([128, 8, CPB], dt.bfloat16, tag="xab", name="xab")
                nc.sync.dma_start(xe8, xe_dram[:, :, c0 : c0 + CPB])
                nc.sync.dma_start(xr8, xr_dram[:, :, c0 : c0 + CPB])
                nc.sync.dma_start(xab, xa_dram[:, :, c0 : c0 + CPB])
                qT = spool.tile([128, 8, CPB], dt.bfloat16, tag="qT")
                kTa = spool.tile([128, 8, CPB], dt.bfloat16, tag="kTa")
                kTr = spool.tile([128, 8, CPB], dt.bfloat16, tag="kTr")
                va = vpool.tile([128, 4, 8, HD], dt.bfloat16, tag="va")
                vr = vpool.tile([128, 4, 8, HD], dt.bfloat16, tag="vr")
                st = dict(xe8=xe8, xr8=xr8, xab=xab, qT=qT, kTa=kTa, kTr=kTr,
                          va=va, vr=vr, c0=c0)
                slabs = [qT, kTa, kTr]
                items = []

                def qkv_item(fp):
                    # computes fc = 2*fp, 2*fp+1 (same slab)
                    def go():
                        ps = mmpool.tile([128, 2, 512], dt.float32, tag="mmps")
                        for half in range(2):
                            fc = 2 * fp + half
                            src = xr8 if fc >= 16 else xe8
                            for j in range(4):
                                nc.tensor.matmul(
                                    ps[:, half, :],
                                    wqkv_sb[:, 2 * j : 2 * j + 2,
                                            fc * 128 : (fc + 1) * 128],
                                    src[:, 2 * j : 2 * j + 2, :],
                                    start=(j == 0),
                                    stop=(j == 3),
                                    perf_mode=DR,
                                )
                        fc0 = 2 * fp
                        dst = slabs[fc0 // 8][:, fc0 % 8 : fc0 % 8 + 2, :]
                        nc.vector.tensor_copy(dst, ps)
                    return go

                def v_item(isart, rc):
                    def go():
                        src, dstv = (xe8, va) if isart else (xr8, vr)
                        ps = mmpool.tile([128, 2, 512], dt.float32, tag="mmps")
                        for vh in range(2):
                            for j in range(4):
                                nc.tensor.matmul(
                                    ps[:, vh, :],
                                    src[:, 2 * j : 2 * j + 2,
                                        rc * 128 : (rc + 1) * 128],
                                    wv_sb[:, 2 * j : 2 * j + 2,
                                          vh * 512 : (vh + 1) * 512],
                                    start=(j == 0),
                                    stop=(j == 3),
                                    perf_mode=DR,
                                )
                        dv = dstv[:, rc, :, :]
                        nc.scalar.copy(out=dv, in_=ps)
                    return go

                for fp in range(12):
                    items.append(qkv_item(fp))
                for isart in (True, False):
                    for rc in range(4):
                        items.append(v_item(isart, rc))
                return st, items

            def build_attn_items(st):
                """Work items for norm + attention + proj of a block."""
                qT, kTa, kTr = st["qT"], st["kTa"], st["kTr"]
                va, vr, xab, c0 = st["va"], st["vr"], st["xab"], st["c0"]
                items = []

                def norm_item(slab, scv, hp):
                    # normalizes head-chunks 2*hp, 2*hp+1 of slab; rsqrt as
                    # exp(-0.5*ln(m)) so ACT never leaves table set 6 (eps is
                    # dropped: it is ~6e-8 relative to ssq and unobservable).
                    def go():
                        with nc.allow_low_precision(
                            reason="all-reduce upcasts internally; bf16 ~0.4%"
                        ):
                            sl = slab[:, 2 * hp : 2 * hp + 2, :]
                            sq = npool.tile([128, 2, CPB], dt.bfloat16, tag="nsq")
                            nc.vector.tensor_mul(sq, sl, sl)
                            ssq = npool.tile([128, 2, CPB], dt.bfloat16, tag="nssq")
                            for hf in range(2):
                                nc.gpsimd.partition_all_reduce(
                                    ssq[:, hf, :], sq[:, hf, :], channels=128,
                                    reduce_op=bass_isa.ReduceOp.add,
                                )
                            lnm = npool.tile([128, 2, CPB], dt.bfloat16, tag="nsq")
                            nc.scalar.activation(lnm, ssq, Act.Ln, scale=scv)
                            rs = npool.tile([128, 2, CPB], dt.bfloat16, tag="nssq")
                            nc.scalar.activation(rs, lnm, Act.Exp, scale=-0.5)
                            nc.vector.tensor_mul(sl, sl, rs)
                    return go

                mixedT = vpool.tile([128, 8, CPB], dt.float8e4, tag="mixedT")
                st["mixedT"] = mixedT
                routes = {}

                def scores_item(g, half):
                    def go():
                        gsl = slice(g * 128, (g + 1) * 128)
                        kT = kTa if half == 0 else kTr
                        ps = scpool.tile([128, 8, 128], dt.float32, tag="scps")
                        for h in range(8):
                            nc.tensor.matmul(
                                ps[:, h, :], kT[:, h, gsl], qT[:, h, gsl],
                                start=True, stop=True,
                            )
                        esc = fpool.tile([128, 8, 128], dt.bfloat16, tag="ers")
                        rsc = fpool.tile([128, 8, 128], dt.bfloat16, tag="ers")
                        nc.scalar.activation(esc, ps, Act.Exp)
                        nc.scalar.activation(rsc, ps, Act.Relu)
                        # elu = relu(s) + (min(exp(s),1) - 1)
                        nc.vector.tensor_scalar(esc, esc, 1.0, -1.0, Alu.min, Alu.add)
                        nc.vector.tensor_add(esc, rsc, esc)
                        route = rtpool.tile([128, 8, 128], dt.bfloat16, tag="rt")
                        nc.vector.tensor_mul(
                            route, esc,
                            mask_sb[:, None, :].to_broadcast((128, 8, 128)),
                        )
                        routes[(g, half)] = route
                    return go

                def mixed_item(g):
                    def go():
                        gsl = slice(g * 128, (g + 1) * 128)
                        mx = scpool.tile([128, 8, 128], dt.float32, tag="scps")
                        for h in range(8):
                            nc.tensor.matmul(
                                mx[:, h, :], va[:, g, h, :],
                                routes[(g, 0)][:, h, :], start=True, stop=False,
                            )
                            nc.tensor.matmul(
                                mx[:, h, :], vr[:, g, h, :],
                                routes[(g, 1)][:, h, :], start=False, stop=True,
                            )
                        nc.scalar.copy(out=mixedT[:, :, gsl], in_=mx)
                    return go

                def proj_item(dp):
                    # projects feature chunks dc = 2*dp, 2*dp+1
                    def go():
                        ps = mmpool.tile([128, 2, 512], dt.float32, tag="mmps")
                        yb = ypool.tile([128, 2, CPB], dt.bfloat16, tag="yb")
                        for half in range(2):
                            dc = 2 * dp + half
                            for j in range(4):
                                nc.tensor.matmul(
                                    ps[:, half, :],
                                    wproj_sb[:, 2 * j : 2 * j + 2,
                                             dc * 128 : (dc + 1) * 128],
                                    mixedT[:, 2 * j : 2 * j + 2, :],
                                    start=(j == 0), stop=(j == 3),
                                    perf_mode=DR,
                                )
                            nc.vector.scalar_tensor_tensor(
                                out=yb[:, half, :], in0=ps[:, half, :],
                                scalar=mscale_sb[:, dc : dc + 1],
                                in1=xab[:, dc, :],
                                op0=Alu.mult, op1=Alu.add,
                            )
                        nc.sync.dma_start(
                            yo_dram[:, 2 * dp : 2 * dp + 2, c0 : c0 + CPB], yb
                        )
                    return go

                norm_items = []
                for slab, scv in (
                    (qT, 1.0 / HD),
                    (kTa, 1.0),
                    (kTr, 1.0),
                ):
                    for hp in range(4):
                        norm_items.append(norm_item(slab, scv, hp))
                return dict(
                    norm=norm_items,
                    groups=[(scores_item(g, 0), scores_item(g, 1), mixed_item(g))
                            for g in range(4)],
                    proj=[proj_item(dp) for dp in range(4)],
                )

            def merge(attn, gemm):
                """Structured interleave: norm 3:1 with gemms, then per group
                sc,G,sc,G,mx,G, then proj 1:1 with gemms; leftovers last."""
                out = []
                gq = list(gemm)

                def g(n):
                    for _ in range(n):
                        if gq:
                            out.append(gq.pop(0))

                if attn is None:
                    return list(gemm)
                for i, it in enumerate(attn["norm"]):
                    out.append(it)
                    if i % 3 == 2:
                        g(1)
                for sc0, sc1, mx in attn["groups"]:
                    out.append(sc0); g(1)
                    out.append(sc1); g(1)
                    out.append(mx); g(1)
                for p in attn["proj"]:
                    out.append(p); g(1)
                out.extend(gq)
                return out

            blklist = [b for _ in range(repeat) for b in range(nb)]
            prev_st = None
            for i in range(len(blklist) + 1):
                gemm_items = []
                if i < len(blklist):
                    st, gemm_items = build_gemm_items(blklist[i])
                attn = build_attn_items(prev_st) if prev_st is not None else None
                for item in merge(attn, gemm_items):
                    item()
                if i < len(blklist):
                    prev_st = st

    nc.compile()
    return nc


def host_prep(x, artery_embed, residual_kv, Wqkv, Wproj, mixer_scale,
              tok_per_core=TOK_PER_CORE, n_cores=N_CORES):
    T = x.shape[0] * x.shape[1]
    x_flat = np.asarray(x, dtype=np.float32).reshape(T, A, DIM)
    res_flat = np.asarray(residual_kv, dtype=np.float32).reshape(T, RKV, DIM)
    emb = np.asarray(artery_embed, dtype=np.float32)
    xe_flat = x_flat + emb[None]

    Rm = _rope_matrix()
    Wq = np.asarray(Wqkv[0:MD], dtype=np.float64)
    Wk = np.asarray(Wqkv[MD : 2 * MD], dtype=np.float64)
    Wv = np.asarray(Wqkv[2 * MD : 3 * MD], dtype=np.float64)
    Wk_res = np.einsum("de,hec->hdc", Rm, Wk.reshape(HEADS, HD, DIM)).reshape(MD, DIM)

    wqkv_t = np.ascontiguousarray(
        np.concatenate([Wq, Wk, Wk_res], axis=0).T * WSCALE
    ).astype(f8e4)
    wv_t = np.ascontiguousarray(Wv.T * WSCALE).astype(f8e4)
    wproj_t = np.ascontiguousarray(
        np.asarray(Wproj, dtype=np.float64).T * WSCALE
    ).astype(f8e4)

    mask = np.zeros((128, 128), dtype=np.float32)
    for t in range(16):
        mask[t * 8 : (t + 1) * 8, t * 8 : (t + 1) * 8] = 1.0 / SC
    mask = mask.astype(bf16)

    # v path and proj each carry WSCALE; descale both via mscale.
    mscale = np.ascontiguousarray(
        (np.asarray(mixer_scale, dtype=np.float32) / (WSCALE * WSCALE))
        .reshape(8, 128).T
    )

    shared = dict(
        wqkv_t=wqkv_t, wv_t=wv_t, wproj_t=wproj_t, mask=mask, mscale=mscale,
    )
    in_maps = []
    for i in range(n_cores):
        sl = slice(i * tok_per_core, (i + 1) * tok_per_core)
        xe = np.ascontiguousarray(
            xe_flat[sl].reshape(tok_per_core * A, DIM).T
        ).astype(f8e4)
        xr = np.ascontiguousarray(
            res_flat[sl].reshape(tok_per_core * RKV, DIM).T
        ).astype(f8e4)
        xa = np.ascontiguousarray(
            x_flat[sl].reshape(tok_per_core * A, DIM).T
        ).astype(bf16)
        m = dict(shared)
        m["xe8_t"] = xe
        m["xr8_t"] = xr
        m["xab_t"] = xa
        in_maps.append(m)
    return in_maps


def assemble_output(outs, tok_per_core=TOK_PER_CORE):
    """outs: list of (DIM, tok_per_core*8) bf16 arrays -> (B,S,A,DIM) f32."""
    parts = []
    for o in outs:
        y = np.asarray(o, dtype=np.float32)  # (1024, T*8)
        parts.append(y.reshape(DIM, tok_per_core, A).transpose(1, 2, 0))
    full = np.concatenate(parts, axis=0)  # (n_tok, A, DIM)
    if full.shape[0] == B * S:
        full = full.reshape(B, S, A, DIM)
    return np.ascontiguousarray(full)


_NC_CACHE = {}


def kernel(x, artery_embed, residual_kv, Wqkv, Wproj, mixer_scale):
    from concourse.bass_utils import run_bass_kernel_spmd

    key = TOK_PER_CORE
    if key not in _NC_CACHE:
        _NC_CACHE[key] = build_program(TOK_PER_CORE)
    nc = _NC_CACHE[key]

    in_maps = host_prep(x, artery_embed, residual_kv, Wqkv, Wproj, mixer_scale)
    res = run_bass_kernel_spmd(nc, in_maps, core_ids=list(range(N_CORES)))
    outs = [r["out_t"] for r in res.results]
    return assemble_output(outs)
